# revision 7
# baseline (speedup 1.0000x reference)
"""Causal multi-head attention layer for Trainium2 (Bass/Tile), 8 NeuronCores.

Problem: x[B=2,S=2048,D=1024], H=16 heads, Dh=64.
Sharding: data-parallel over batch (2) x tensor-parallel over head groups (4):
each of the 8 cores handles one batch element and 4 heads, producing a partial
output [S, D]; the host sums the 4 head-group partials per batch (the
"all-reduce after the W_O contraction" done host-side since we return full
output anyway) and adds biases that commute out (b_O and sum_h b_V[h] @ W_O[h],
exact because softmax rows sum to 1).

Device kernel (per core). The SCORE path (QK projections + S=Q.K^T) runs in
fp8e4m3 with MatmulPerfMode.DoubleRow (2 fp8 MACs/PE-cell/cycle = 2x f16
matmul throughput, measured 216ns for K_eff=256,N=512 — same as one f16
K=128 matmul); the VALUE path (V projection, PV, output projection) stays
f16: fp8 quantization error on the score path averages out through softmax
(host-sim rel_absmax ~1.4e-2 vs the 2e-2 gate), but value-path fp8 error
(~3.6% rms) passes straight to the output and would fail.

  - x is fed twice: f16 x^T [128, KT=8, S] (V path) and fp8 x8
    [128, 2, KT2=4, S] with D-pairs packed in the DoubleRow slot dim
    (d = kt2*256 + slot*128 + p).
  - W_Q/W_K are host-packed fp8 at 64x scale (keeps the 0.02-std weights
    out of e4m3 denormals), wqk8 [128, 2, KT2, 2, NPAIR, 128]. A QK
    projection group is 4 DoubleRow matmuls (K_eff=256 each) instead of 8
    f16 ones. PSUM holds 64*q; eviction applies *1/16 (+4*bias) and writes
    fp8 Q8/K8 at 4x scale (sigma~2.6, e4m3-friendly); the *16 in the score
    product is folded into the exp scale (inv_sqrt_dh/16).
  - Scores matmul per j is a DoubleRow PAIR: head A packed [32,2] (e =
    slot*32 + p) at PE rows 0:32, head B at rows 64:96. Rows {0,64} are
    different PE quadrants so the two matmuls run fully concurrent
    (measured): one N-cycle pass for both heads vs ~1.7N for the old f16
    K=64 pair trick. Operand base partitions are restricted to {0,32,64}
    and quadrant concurrency needs {0,64}, so only 2 heads can fly at once.
  - Q8/K8 eviction cannot write the [32,2]-packed layout directly (it folds
    64 PSUM partitions onto 32) — evict full-width [128,SC] fp8 scratch
    (one DVE/ACT op, same cost as the old f16 eviction), then 4 tiny
    SBUF->SBUF DMAs fold it into Q8/K8. DMA queues are idle mid-kernel.
  - V computed in [k, e] layout from f16 x^T (stationary x^T tile, moving
    W_V, all 4 heads at once), stored as V'=[V|1...1] with the ones block
    replicated 64x so the PV matmul broadcasts the softmax denominator l
    across output partitions 64:128.
  - Scores computed TRANSPOSED: S^T[k, q], so softmax's sum rides the PV
    contraction: Z'[e|l, q] = V'.T @ exp(S^T) accumulated over k-tiles. No
    max-subtraction (scores are O(1), exp safe in f32).
  - Causal masking is multiplicative on exp(S^T), diagonal chunks only (on
    GpSimd); fully-masked column ranges are skipped via c0 slicing.
  - Normalization: l arrives pre-broadcast on PV-accumulator partitions
    64:128; wide DVE reciprocal_approx_fast + multiply. (Must stage l
    through SBUF — reciprocal on multi-matmul PSUM directly is garbage.)
  - Schedule (all tuned on HW, see git history of the f16 version):
      * Phase 1 computes only the first two q-chunks' Q/K projections
        (8 PSUM groups fed kt2-by-kt2 as the x8 DMA lands) and the first
        8 V tiles; the rest ride the flash loop as deadline-ordered PE
        fill work (fill_queue).
      * DMA order: bqk, wqk8+x8[chunks 0-1 cols] interleaved, f16 x^T
        ktile-by-ktile with wv at midpoint, x8[chunks 2-3 cols], wo.
      * exp->PV pipeline depth 2 (pends); out-proj METERED (every 3rd j)
        through middle chunks; pr-boundary cover steps; drain-phase
        normalize sliced per q-tile with out-proj interleaved.
      * Out-proj PSUM evicted on DVE during flash, ACT during drain; casts
        to f16 so the out DMA halves (host accumulates partials in f32).
  - CAUTION: instruction timings are extremely sensitive to SBUF tile
    layout (port contention). A/B any pool/tile change against the
    previous layout.
"""

import os
import numpy as np

P = 128
SC = 512  # q-chunk width (one PSUM bank of fp32)

_BUILD_CACHE = {}

WSCALE = 64.0   # host scale on W_Q/W_K before fp8 quantization
QSCALE = 4.0    # scale of Q8/K8 relative to true q,k
# eviction: psum = WSCALE * q  ->  Q8 = psum * (QSCALE/WSCALE) + QSCALE*b
EVSCALE = QSCALE / WSCALE
# score psum = QSCALE^2 * (q.k); fold into exp scale
SSCALE = 1.0 / (QSCALE * QSCALE)


def build_nc(S, Dm, NH, Dh, stage=99):
    """Build (and cache) the per-core Bass module. NH = heads per core."""
    key = (S, Dm, NH, Dh, stage)
    if key in _BUILD_CACHE:
        return _BUILD_CACHE[key]

    import concourse.bacc as bacc
    import concourse.mybir as mybir
    import concourse.tile as tile

    f32 = mybir.dt.float32
    f16 = mybir.dt.float16
    f8 = mybir.dt.float8e4
    DR = mybir.MatmulPerfMode.DoubleRow
    dt_w = f16   # value-path matmul dtype
    dt_m = f16   # mask dtype

    KT = Dm // P       # f16 k-tiles over the model dim
    KT2 = Dm // (2 * P)  # fp8 DoubleRow k-tiles (256 contraction each)
    NPAIR = NH // 2    # head pairs
    QC = S // SC       # q chunks
    NKT = S // P       # k-position tiles
    DH2 = Dm // SC     # output free-dim chunks
    assert Dh == 64 and NH % 2 == 0 and S % SC == 0 and Dm % SC == 0

    nc = bacc.Bacc(
        "TRN2",
        debug=False,
        enable_asserts=False,
        target_bir_lowering=False,
        num_devices=1,
    )

    xT_d = nc.dram_tensor("xT", [P, KT, S], f16, kind="ExternalInput")
    x8_d = nc.dram_tensor("x8", [P, 2, KT2, S], f8, kind="ExternalInput")
    wqk8_d = nc.dram_tensor(
        "wqk8", [P, 2, KT2, 2, NPAIR, P], f8, kind="ExternalInput"
    )
    wv_d = nc.dram_tensor("wv", [P, KT, NH * Dh], f16, kind="ExternalInput")
    wo_d = nc.dram_tensor("wo", [P, NPAIR, Dm], f16, kind="ExternalInput")
    bqk_d = nc.dram_tensor("bqk", [P, 2, NPAIR], f32, kind="ExternalInput")
    # output in f16 (halves the output DMA; host accumulates in f32)
    out_d = nc.dram_tensor("out", [S, Dm], f16, kind="ExternalOutput")

    Exp = mybir.ActivationFunctionType.Exp
    Ident = mybir.ActivationFunctionType.Identity
    exp_scale = float(SSCALE / np.sqrt(Dh))

    with tile.TileContext(nc) as tc:
        with tc.tile_pool(name="const", bufs=1) as cpool:
            wv = cpool.tile([P, KT, NH * Dh], f16)
            wo = cpool.tile([P, NPAIR, Dm], f16)
            bqk = cpool.tile([P, 2, NPAIR], f32)

            # fp8 Q/K in DoubleRow-packed layout: partition p in [0,32) +
            # slot s hold head A's e = s*32+p; partitions 64:96 head B.
            # (32:64 and 96:128 are dead — operand bases are {0,32,64} and
            # quadrant concurrency needs {0,64}.)
            Q8 = cpool.tile([P, 2, NPAIR, S], f8)
            K8 = cpool.tile([P, 2, NPAIR, S], f8)
            Vt = cpool.tile([P, NKT, NH, 2 * Dh], f16)

            # causal masks for the diagonal-chunk variants (S^T layout:
            # partition=k, free=q), built on GpSimd during the DMA wait
            masks = cpool.tile([P, SC // P, SC], dt_m)
            nc.gpsimd.memset(masks[:], 1.0)
            for v in range(SC // P):
                nc.gpsimd.affine_select(
                    out=masks[:, v, :],
                    in_=masks[:, v, :],
                    compare_op=mybir.AluOpType.is_ge,
                    fill=0.0,
                    base=-(v * P),
                    pattern=[[1, SC]],
                    channel_multiplier=-1,
                )

            # ---------- phase 1: projections for the first two q-chunks ----
            with (
                tc.tile_pool(name="p1", bufs=1) as p1pool,
                tc.tile_pool(name="ps1", bufs=8, space="PSUM") as ps1,
            ):
                wqk8 = cpool.tile([P, 2, KT2, 2, NPAIR, P], f8)
                x8 = cpool.tile([P, 2, KT2, S], f8)
                xT = cpool.tile([P, KT, S], f16)
                # DMA order: the fp8 QK stream first (it gates the flash
                # start), then the f16 x for the V path, then the deferred
                # x8 columns (feed the in-flash qk fills), then wo.
                nc.sync.dma_start(bqk[:], bqk_d[:])
                for kt2 in range(KT2):
                    nc.sync.dma_start(wqk8[:, :, kt2], wqk8_d[:, :, kt2])
                    nc.sync.dma_start(
                        x8[:, :, kt2, 0 : 2 * SC], x8_d[:, :, kt2, 0 : 2 * SC]
                    )
                for kt in range(KT):
                    nc.sync.dma_start(xT[:, kt, :], xT_d[:, kt, :])
                    if kt == KT // 2 - 1:
                        nc.sync.dma_start(wv[:], wv_d[:])
                for kt2 in range(KT2):
                    nc.sync.dma_start(
                        x8[:, :, kt2, 2 * SC : S], x8_d[:, :, kt2, 2 * SC : S]
                    )
                nc.sync.dma_start(wo[:], wo_d[:])

                # HAM warm-up: dummy matmuls during the initial DMA wait so
                # the PE clock-gate is at 8/8 when real work arrives
                wst = p1pool.tile([P, SC], f32)
                nc.vector.memset(wst[:], 1.0)
                # preload the Exp table on the Scalar engine now (idle)
                tpre = p1pool.tile([1, 2], f32)
                nc.scalar.activation(tpre[:], wst[0:1, 0:2], Exp)
                wrm = p1pool.tile([P, SC], f16)
                nc.vector.tensor_copy(wrm[:], wst[:])
                nwu = 10
                pwu = ps1.tile([P, SC], f32, tag="mm")
                for i in range(nwu):
                    nc.tensor.matmul(
                        pwu[:], wrm[:, 0:P], wrm[:],
                        start=(i == 0), stop=(i == nwu - 1),
                    )

                def fold_qk(u8, pj, pr, qc):
                    """4 SBUF->SBUF DMAs: unpacked fp8 [128,SC] eviction ->
                    DoubleRow-packed Q8/K8 slices."""
                    dst = Q8 if pj == 0 else K8
                    qs = slice(qc * SC, (qc + 1) * SC)
                    for base in (0, 64):
                        for s in (0, 1):
                            src = u8[base + 32 * s : base + 32 * s + 32, :]
                            nc.sync.dma_start(
                                dst[base : base + 32, s, pr, qs], src
                            )

                # Q/K projections (first two q-chunks): 8 PSUM groups fed
                # kt2-by-kt2 as the x8 DMA lands
                for qg in range(0, min(2, QC), 2):
                    qcs = list(range(qg, min(qg + 2, QC)))
                    pss = {
                        (pr, pj, qc): ps1.tile(
                            [P, SC], f32, tag="mm", name=f"psqk_{pr}_{pj}_{qc}"
                        )
                        for pr in range(NPAIR)
                        for pj in range(2)
                        for qc in qcs
                    }
                    for kt2 in range(KT2):
                        st, sp = kt2 == 0, kt2 == KT2 - 1
                        for pr in range(NPAIR):
                            for pj in range(2):
                                for qc in qcs:
                                    xs = x8[:, :, kt2, qc * SC : (qc + 1) * SC]
                                    nc.tensor.matmul(
                                        pss[(pr, pj, qc)][:],
                                        wqk8[:, :, kt2, pj, pr, :], xs,
                                        start=st, stop=sp, perf_mode=DR,
                                    )
                    for pr in range(NPAIR):
                        for qc in qcs:
                            for pj in range(2):
                                # evict via ACT (idle in phase 1; Identity
                                # shares the Exp table) to full-width fp8
                                # scratch, then DMA-fold into Q8/K8
                                u8 = p1pool.tile(
                                    [P, SC], f8, name=f"u8_{pr}_{pj}_{qc}"
                                )
                                nc.scalar.activation(
                                    u8[:], pss[(pr, pj, qc)][:], Ident,
                                    bias=bqk[:, pj, pr : pr + 1],
                                    scale=EVSCALE,
                                )
                                fold_qk(u8, pj, pr, qc)

                # V tiles the first two flash chunks touch; rest deferred
                for qt in range(min(2 * (SC // P), NKT)):
                    psV = ps1.tile([P, NH * Dh], f32, tag="mm")
                    for kt in range(KT):
                        nc.tensor.matmul(
                            psV[:],
                            xT[:, kt, qt * P : (qt + 1) * P],
                            wv[:, kt, :],
                            start=(kt == 0), stop=(kt == KT - 1),
                        )
                    nc.vector.tensor_copy(
                        Vt[:, qt, :, 0:Dh],
                        psV[:].rearrange("p (h e) -> p h e", e=Dh),
                    )

                # V' ones block (broadcasts l onto PV partitions 64:128)
                cstage = p1pool.tile([P, 1, 1, Dh], f32)
                nc.vector.memset(cstage[:], 1.0)
                nc.vector.tensor_copy(
                    Vt[:, :, :, Dh : 2 * Dh],
                    cstage[:].to_broadcast((P, NKT, NH, Dh)),
                )

            # ---------- phases 2+3 ----------
            with tc.tile_pool(name="zt", bufs=1) as ztpool:
                ZTt = ztpool.tile([P, NPAIR, S], f16)
                self_flash(
                    nc, tc, stage, Exp, exp_scale, mybir,
                    Q8, K8, Vt, ZTt, wo, out_d, masks, xT, x8, wv, wqk8, bqk,
                    S, Dm, Dh, NPAIR, QC, SC, P, DH2, KT, KT2, NKT,
                    f16, dt_m, f32, f8, DR,
                )

    nc.compile()
    _BUILD_CACHE[key] = nc
    return nc


def self_flash(
    nc, tc, stage, Exp, exp_scale, mybir,
    Q8, K8, Vt, ZTt, wo, out_d, masks, xT, x8, wv, wqk8, bqk,
    S, Dm, Dh, NPAIR, QC, SC, P, DH2, KT, KT2, NKT,
    dt_w, dt_m, f32, f8, DR,
):
    NH = Vt.shape[2]
    # ---------- phases 2+3: flash attention (scores transposed, fp8
    # DoubleRow) with the output projection interleaved one q-chunk behind
    out_dt = dt_w
    mult, add = mybir.AluOpType.mult, mybir.AluOpType.add
    with (
        tc.tile_pool(name="e", bufs=4) as epool,
        tc.tile_pool(name="r", bufs=4) as rpool,
        tc.tile_pool(name="o", bufs=4) as opool,
        tc.tile_pool(name="pss", bufs=2, space="PSUM") as ps_s,
        tc.tile_pool(name="psz", bufs=4, space="PSUM") as psz,
    ):
        if stage <= 1:
            nc.sync.dma_start(out_d[0:P, :], ZTt[:, 0, 0:Dm])

        drain = [False]  # final-drain mode: outproj evictions move DVE->ACT

        def normalize(pr, qc, zA, zB):
            """ZT[:, q] = Z'[0:64, q] * (1 / l[q]); l arrives pre-broadcast
            on partitions 64:128 of the PV accumulators. DVE-only."""
            qs = slice(qc * SC, (qc + 1) * SC)
            rb = rpool.tile([64, 2, SC], f32, tag="rb")
            ls = rpool.tile([64, 2, SC], f32, tag="ls")
            nc.vector.tensor_copy(ls[:, 0, :], zA[Dh : 2 * Dh, :])
            nc.vector.tensor_copy(ls[:, 1, :], zB[Dh : 2 * Dh, :])
            nc.vector.reciprocal_approx_fast(rb[:], ls[:])
            nc.vector.tensor_mul(ZTt[0:64, pr, qs], zA[0:Dh, :], rb[:, 0, :])
            nc.vector.tensor_mul(ZTt[64:128, pr, qs], zB[0:Dh, :], rb[:, 1, :])

        def outproj_steps(qc):
            """Closures for this q-chunk's output projection, injected one at
            a time between later j-iterations to keep PE density high."""
            def step(t, dh2):
                def emit():
                    po = psz.tile([P, SC], f32, tag="z")
                    ds = slice(dh2 * SC, (dh2 + 1) * SC)
                    zs = slice(t * P, (t + 1) * P)
                    for pr in range(NPAIR):
                        nc.tensor.matmul(
                            po[:], ZTt[:, pr, zs], wo[:, pr, ds],
                            start=(pr == 0), stop=(pr == NPAIR - 1),
                        )
                    ot = opool.tile([P, SC], out_dt, tag="o")
                    # evict via DVE during flash, ACT during the final drain
                    if drain[0]:
                        nc.scalar.activation(
                            ot[:], po[:], mybir.ActivationFunctionType.Copy
                        )
                    else:
                        nc.vector.tensor_copy(ot[:], po[:])
                    nc.sync.dma_start(out_d[t * P : (t + 1) * P, ds], ot[:])
                return emit

            return [
                step(t, dh2)
                for t in range(qc * (SC // P), (qc + 1) * (SC // P))
                for dh2 in range(DH2)
            ]

        def v_step(qt):
            """One deferred V-projection group (f16): PE fill work."""
            def emit():
                psV = psz.tile([P, NH * Dh], f32, tag="z", name=f"psv_{qt}")
                for kt in range(KT):
                    nc.tensor.matmul(
                        psV[:],
                        xT[:, kt, qt * P : (qt + 1) * P],
                        wv[:, kt, :],
                        start=(kt == 0), stop=(kt == KT - 1),
                    )
                nc.vector.tensor_copy(
                    Vt[:, qt, :, 0:Dh],
                    psV[:].rearrange("p (h e) -> p h e", e=Dh),
                )
            return emit

        def qk_step(qc, pr, pj):
            """One deferred Q/K-projection group: 4 fp8 DoubleRow matmuls,
            DVE eviction to fp8 scratch, DMA-fold into Q8/K8."""
            def emit():
                ps = psz.tile([P, SC], f32, tag="z", name=f"psqk{qc}_{pr}_{pj}")
                qs = slice(qc * SC, (qc + 1) * SC)
                for kt2 in range(KT2):
                    nc.tensor.matmul(
                        ps[:], wqk8[:, :, kt2, pj, pr, :],
                        x8[:, :, kt2, qs],
                        start=(kt2 == 0), stop=(kt2 == KT2 - 1), perf_mode=DR,
                    )
                u8 = rpool.tile([P, SC], f8, tag="u8")
                nc.vector.tensor_scalar(
                    u8[:], ps[:], EVSCALE, bqk[:, pj, pr : pr + 1], mult, add
                )
                dst = Q8 if pj == 0 else K8
                for base in (0, 64):
                    for s in (0, 1):
                        nc.sync.dma_start(
                            dst[base : base + 32, s, pr, qs],
                            u8[base + 32 * s : base + 32 * s + 32, :],
                        )
            return emit

        fill_queue = []
        for qc2 in range(2, QC):
            for pr2 in range(NPAIR):
                for pj2 in range(2):
                    fill_queue.append((qc2, qk_step(qc2, pr2, pj2)))
            for qt in range(qc2 * (SC // P), (qc2 + 1) * (SC // P)):
                fill_queue.append((qc2, v_step(qt)))
        op_queue = []
        chunk_tail = None
        for qc in range(QC if stage >= 2 else 0):
            while fill_queue and fill_queue[0][0] <= qc:
                fill_queue.pop(0)[1]()
            for pr in range(NPAIR):
                hA, hB = 2 * pr, 2 * pr + 1
                zA = psz.tile([P, SC], f32, tag="z")
                zB = psz.tile([P, SC], f32, tag="z")
                jmax = (qc + 1) * (SC // P)
                pends = []  # exp->PV pipeline, depth 2

                def emit_pv(j, eAB, c0, jmax=jmax, zA=zA, zB=zB, hA=hA, hB=hB):
                    st, sp = j == 0, j == jmax - 1
                    cs = slice(c0, SC)
                    nc.tensor.matmul(
                        zA[:, cs], Vt[:, j, hA, :], eAB[:, 0, cs],
                        start=st, stop=sp,
                    )
                    nc.tensor.matmul(
                        zB[:, cs], Vt[:, j, hB, :], eAB[:, 1, cs],
                        start=st, stop=sp,
                    )

                for j in range(jmax):
                    v = j - (jmax - SC // P)
                    c0 = v * P if v > 0 else 0
                    cs = slice(c0, SC)
                    qf = slice(qc * SC + c0, (qc + 1) * SC)
                    sAB = ps_s.tile([P, 2, SC], f32, tag="s")
                    ks = slice(j * P, (j + 1) * P)
                    # fp8 DoubleRow score pair: head A rows 0:32, head B
                    # rows 64:96 (different PE quadrants -> concurrent)
                    nc.tensor.matmul(
                        sAB[:, 0, cs],
                        K8[0:32, :, pr, ks], Q8[0:32, :, pr, qf],
                        start=True, stop=True, perf_mode=DR,
                    )
                    nc.tensor.matmul(
                        sAB[:, 1, cs],
                        K8[64:96, :, pr, ks], Q8[64:96, :, pr, qf],
                        start=True, stop=True, perf_mode=DR,
                    )
                    eAB = epool.tile([P, 2, SC], dt_w, tag="e")
                    nc.scalar.activation(
                        eAB[:, :, cs], sAB[:, :, cs], Exp, scale=exp_scale
                    )
                    if v >= 0:  # chunk contains the causal diagonal
                        mv = slice(c0, min((v + 1) * P, SC))
                        nc.gpsimd.tensor_mul(
                            eAB[:, 0, mv], eAB[:, 0, mv], masks[:, v, mv]
                        )
                        nc.gpsimd.tensor_mul(
                            eAB[:, 1, mv], eAB[:, 1, mv], masks[:, v, mv]
                        )
                    if stage >= 3:
                        pends.append((j, eAB, c0))
                        if len(pends) > 3:
                            emit_pv(*pends.pop(0))
                        if j == 0 and chunk_tail is not None:
                            chunk_tail()
                            chunk_tail = None
                        elif j >= 1 and fill_queue:
                            fill_queue.pop(0)[1]()
                        elif j >= 2 and op_queue and (
                            qc == QC - 1 or j % 3 == 0
                        ):
                            op_queue.pop(0)()
                    else:
                        last_e = eAB
                if stage < 3:
                    if pr == 0 and qc == 0:
                        nc.sync.dma_start(out_d[0:P, 0:SC], last_e[:, 0, :])
                    continue

                # pr-boundary cover
                if op_queue and qc >= 1:
                    op_queue.pop(0)()

                def chunk_tail(pends=pends, pr=pr, qc=qc, zA=zA, zB=zB,
                               emit_pv=emit_pv):
                    for p in pends:
                        emit_pv(*p)
                    normalize(pr, qc, zA, zB)
                    return pr, qc, zA, zB

            if stage >= 5:
                op_queue.extend(outproj_steps(qc))
        drain[0] = True
        if chunk_tail is not None:
            pends, pr, qc, zA, zB = (chunk_tail.__defaults__[:5])
            for p in pends:
                chunk_tail.__defaults__[5](*p)
            rb = rpool.tile([64, 2, SC], f32, tag="rb")
            ls = rpool.tile([64, 2, SC], f32, tag="ls")
            for ti in range(SC // P):
                cl = slice(ti * P, (ti + 1) * P)
                qsl = slice(qc * SC + ti * P, qc * SC + (ti + 1) * P)
                nc.vector.tensor_copy(ls[:, 0, cl], zA[Dh : 2 * Dh, cl])
                nc.vector.tensor_copy(ls[:, 1, cl], zB[Dh : 2 * Dh, cl])
                nc.vector.reciprocal_approx_fast(rb[:, :, cl], ls[:, :, cl])
                nc.vector.tensor_mul(ZTt[0:64, pr, qsl], zA[0:Dh, cl], rb[:, 0, cl])
                nc.vector.tensor_mul(ZTt[64:128, pr, qsl], zB[0:Dh, cl], rb[:, 1, cl])
                for _ in range(DH2):
                    if op_queue:
                        op_queue.pop(0)()
        for step in op_queue:
            step()
        if stage == 4:
            nc.sync.dma_start(out_d[0:P, :], ZTt[:, 0, 0:Dm])


def pack_inputs(x_b, W_Q, W_K, W_V, W_O, b_Q, b_K, hds):
    """Host-side packing of one core's shard into the kernel's layouts."""
    import ml_dtypes

    f8 = ml_dtypes.float8_e4m3
    Dm, Dh = W_Q.shape[1], W_Q.shape[2]
    S = x_b.shape[0]
    NH = len(hds)
    NPAIR = NH // 2
    KT = Dm // P
    KT2 = Dm // (2 * P)

    xb = np.asarray(x_b, np.float32)
    xT = np.ascontiguousarray(
        xb.T.reshape(KT, P, S).transpose(1, 0, 2)
    ).astype(np.float16)
    # fp8 x for the QK path: d = kt2*256 + slot*128 + p
    x8 = np.ascontiguousarray(
        xb.T.reshape(KT2, 2, P, S).transpose(2, 1, 0, 3)
    ).astype(f8)

    def pack_w8(W):  # [H, Dm, Dh] -> [P, 2, KT2, NPAIR, 128] fp8 at 64x
        W4 = np.asarray(W, np.float32)[hds]  # [NH, Dm, Dh]
        t = W4.reshape(NPAIR, 2, KT2, 2, P, Dh).transpose(4, 3, 2, 0, 1, 5)
        return (WSCALE * t.reshape(P, 2, KT2, NPAIR, 2 * Dh))

    wqk8 = np.ascontiguousarray(
        np.stack([pack_w8(W_Q), pack_w8(W_K)], axis=3)  # [P,2,KT2,2,NPAIR,128]
    ).astype(f8)

    WV4 = np.asarray(W_V, np.float32)[hds]
    wv = np.ascontiguousarray(
        WV4.reshape(NH, KT, P, Dh).transpose(2, 1, 0, 3).reshape(P, KT, NH * Dh)
    ).astype(np.float16)

    WO4 = np.asarray(W_O, np.float32)[hds]
    wo = np.ascontiguousarray(
        WO4.reshape(NPAIR, 2, Dh, Dm).transpose(1, 2, 0, 3).reshape(P, NPAIR, Dm)
    ).astype(np.float16)

    def pack_b(b):  # [H, Dh] -> [P, NPAIR], pre-scaled by QSCALE
        b4 = np.asarray(b, np.float32)[hds]
        return QSCALE * b4.reshape(NPAIR, 2, Dh).transpose(1, 2, 0).reshape(P, NPAIR)

    bqk = np.ascontiguousarray(
        np.stack([pack_b(b_Q), pack_b(b_K)], axis=1)  # [P, 2, NPAIR]
    ).astype(np.float32)

    return {"xT": xT, "x8": x8, "wqk8": wqk8, "wv": wv, "wo": wo, "bqk": bqk}


def kernel(x, W_Q, W_K, W_V, W_O, b_Q, b_K, b_V, b_O, _trace=False):
    from concourse.bass_utils import run_bass_kernel_spmd

    x = np.asarray(x, np.float32)
    B, S, Dm = x.shape
    H, _, Dh = W_Q.shape
    NCORES = 8
    GB = NCORES // B        # head groups per batch element
    NH = H // GB            # heads per core

    nc = build_nc(S, Dm, NH, Dh)

    in_maps = []
    for c in range(NCORES):
        b, g = c // GB, c % GB
        hds = list(range(g * NH, (g + 1) * NH))
        in_maps.append(
            pack_inputs(x[b], W_Q, W_K, W_V, W_O, b_Q, b_K, hds)
        )

    try:
        res = run_bass_kernel_spmd(
            nc, in_maps, core_ids=list(range(NCORES)), trace=_trace
        )
    except Exception:
        # transient device hiccups usually clear on retry
        res = run_bass_kernel_spmd(
            nc, in_maps, core_ids=list(range(NCORES)), trace=_trace
        )

    out = np.zeros((B, S, Dm), np.float32)
    for c in range(NCORES):
        out[c // GB] += res.results[c]["out"]

    # biases that commute out of the device kernel (softmax rows sum to 1)
    corr = np.asarray(b_O, np.float32) + np.einsum(
        "he,hed->d",
        np.asarray(b_V, np.float32),
        np.asarray(W_O, np.float32),
    )
    out += corr[None, None, :]

    if _trace:
        kernel.last_results = res
    return out
